# revision 1
# baseline (speedup 1.0000x reference)
"""Trainium2 Bass kernel for nn_AutoPruneNet (MLP policy/baseline heads + sampling).

Math (per row r of TB = T*B rows):
    h1 = relu(x @ W1.T + b1)            x: [512], h1: [400]
    h2 = relu(h1 @ W2.T + b2)           h2: [300]
    core = [h2, clip(reward,-1,1), last_action]   [302]
    pl = sigmoid(core @ Wp.T + bp)      [2]  (mu, sigma)
    baseline = core @ Wb.T + bb         [1]
    action = pl0 + pl1 * eps
    out[r] = [pl0, pl1, baseline, action]

Distribution: pure data parallel, TB rows split contiguously across 8 cores
(16384 rows each); weights replicated.

Device layout: activations stay feature-major ("transposed"): [feature, row],
so the contraction dim of every matmul sits on SBUF partitions and no on-chip
transposes are needed. The host pre-transposes the frame once and the output
back once. SBUF access patterns must start at partition 0/32/64/96, so:
  - the three head outputs are spread to psum partitions 0/32/64 via
    zero-padded head-weight columns, then moved to partition 0 by the ACT
    engine (which tolerates differing in/out partition bases);
  - [clip(reward); last_action] ride at partitions 96/97 of the last fc2
    output chunk (rows 44..95 zeroed), so the head contraction needs no
    extra matmul stream.
"""
import sys
import types

import numpy as np
import ml_dtypes

import concourse.bacc as bacc
import concourse.bass as bass
import concourse.mybir as mybir
import concourse.tile as tile
from concourse.bass import ds, ts
from concourse.bass_utils import run_bass_kernel_spmd


def _install_ntff_hook_shim():
    """Provide the optional antenv.axon_hooks module if the image lacks it,
    so a BASS_TRACE env var in the caller can't crash run_bass_kernel_spmd.
    Registers the real NTFF profile hook when the axon .so supports it."""
    try:
        import antenv.axon_hooks  # noqa: F401
        return
    except Exception:
        pass
    try:
        import antenv
    except Exception:
        return
    mod = types.ModuleType("antenv.axon_hooks")
    state = {"hook": None}
    mod.set_axon_ntff_profile_hook = lambda h: state.__setitem__("hook", h)
    mod.get_axon_ntff_profile_hook = lambda: state["hook"]
    sys.modules["antenv.axon_hooks"] = mod
    antenv.axon_hooks = mod
    try:
        from trn_agent_boot.trn_boot import _ntff_profile_via_ctypes
        mod.set_axon_ntff_profile_hook(
            _ntff_profile_via_ctypes('/opt/axon/libaxon_pjrt.so'))
    except Exception:
        pass


_install_ntff_hook_shim()

BF16 = ml_dtypes.bfloat16

N_CORES = 8
T, B, OBS = 64, 2048, 512
H1, H2 = 400, 300
TB = T * B
R = TB // N_CORES       # rows per core
NT = 512                # rows per row-tile (matmul moving dim)
OG = 4                  # row-tiles per output-DMA group

F32 = mybir.dt.float32
BF = mybir.dt.bfloat16
AF = mybir.ActivationFunctionType
ALU = mybir.AluOpType

# fc1 output (h1) chunking (also fc2 contraction chunking)
M1 = [(0, 100), (100, 100), (200, 100), (300, 100)]
# fc2 output (h2) chunking: {128, 128, 44}; chunk 2 padded to 98 rows with
# zeros at 44..95 and [cr; la] at 96..97
M2 = [(0, 128), (128, 128), (256, 44)]


def build_bass(rows: int):
    """Build the per-core Bass program for `rows` rows (rows % (NT*OG) == 0)."""
    assert rows % (NT * OG) == 0
    n_tiles = rows // NT

    nc = bacc.Bacc("TRN2", target_bir_lowering=False, debug=False)

    xt_d = nc.dram_tensor("xt", [128, 4, rows], BF, kind="ExternalInput")
    rwla_d = nc.dram_tensor("rwla", [2, rows], BF, kind="ExternalInput")
    eps_d = nc.dram_tensor("eps", [1, rows], F32, kind="ExternalInput")
    w1_d = nc.dram_tensor("w1", [128, 4, 400], BF, kind="ExternalInput")
    w2_d = nc.dram_tensor("w2", [100, 4, 300], BF, kind="ExternalInput")
    wh_d = nc.dram_tensor("wh", [128, 3, 65], BF, kind="ExternalInput")
    b1_d = nc.dram_tensor("b1", [100, 4], F32, kind="ExternalInput")
    b2_d = nc.dram_tensor("b2", [128, 3], F32, kind="ExternalInput")
    bh_d = nc.dram_tensor("bh", [65, 1], F32, kind="ExternalInput")
    out_d = nc.dram_tensor("out", [4, rows], F32, kind="ExternalOutput")

    with tile.TileContext(nc) as tc:
        with (
            tc.tile_pool(name="w", bufs=1) as wpool,
            tc.tile_pool(name="x", bufs=3) as xpool,
            tc.tile_pool(name="h1", bufs=8) as h1pool,
            tc.tile_pool(name="core", bufs=8) as cpool,
            tc.tile_pool(name="s", bufs=4) as spool,
            tc.tile_pool(name="ob", bufs=2) as opool,
            tc.tile_pool(name="ps1", bufs=4, space="PSUM") as ppool1,
            tc.tile_pool(name="ps2", bufs=2, space="PSUM") as ppool2,
            tc.tile_pool(name="ps3", bufs=2, space="PSUM") as ppool3,
        ):
            w1_sb = wpool.tile([128, 4, 400], BF, tag="w1")
            nc.scalar.dma_start(w1_sb[:], w1_d[:])
            w2_sb = wpool.tile([100, 4, 300], BF, tag="w2")
            nc.scalar.dma_start(w2_sb[:], w2_d[:])
            wh_sb = wpool.tile([128, 3, 65], BF, tag="wh")
            nc.scalar.dma_start(wh_sb[:], wh_d[:])
            b1_sb = wpool.tile([100, 4, 1], F32, tag="b1")
            nc.scalar.dma_start(b1_sb[:], b1_d[:])
            b2_sb = wpool.tile([128, 3, 1], F32, tag="b2")
            nc.scalar.dma_start(b2_sb[:], b2_d[:])
            bh_sb = wpool.tile([65, 1], F32, tag="bh")
            nc.scalar.dma_start(bh_sb[:], bh_d[:])

            # Software pipeline: the head matmuls + epilogue of tile t-1 are
            # emitted between fc1(t) and fc2(t), so fc2's matmuls get three
            # extra streams of slack for the fc1 relus to land (profiling
            # showed fc2 stalling ~1.2us on the relu semaphore otherwise).
            obs = {}        # group -> (ob tile, et8 tile)
            pending = None  # (cores, t) awaiting head + epilogue

            def emit_head_epilogue(cores, t):
                g, ti = divmod(t, OG)
                ob, et8 = obs[g]
                # heads: psum rows 0=mu_pre, 32=sigma_pre, 64=baseline_pre
                psh = ppool3.tile([65, NT], F32, tag="ps3")
                nc.tensor.matmul(psh[:], wh_sb[0:98, 2, :], cores[2][:],
                                 start=True, stop=False)
                nc.tensor.matmul(psh[:], wh_sb[:, 0, :], cores[0][:],
                                 start=False, stop=False)
                nc.tensor.matmul(psh[:], wh_sb[:, 1, :], cores[1][:],
                                 start=False, stop=True)
                # epilogue — ACT moves rows 32/64 down to partition 0
                sl = ds(ti * NT, NT)
                nc.scalar.activation(ob[:, 0, sl], psh[0:1, :],
                                     AF.Sigmoid, bias=bh_sb[0:1, :])
                nc.scalar.activation(ob[:, 1, sl], psh[32:33, :],
                                     AF.Sigmoid, bias=bh_sb[32:33, :])
                nc.scalar.activation(ob[:, 2, sl], psh[64:65, :],
                                     AF.Identity, bias=bh_sb[64:65, :])
                se = spool.tile([1, NT], F32, tag="se")
                nc.vector.tensor_mul(se[:], ob[:, 1, sl], et8[:, sl])
                nc.vector.tensor_add(ob[:, 3, sl], ob[:, 0, sl], se[:])
                if ti == OG - 1:
                    nc.sync.dma_start(out_d[0:4, ts(g, OG * NT)], ob[:])
                    del obs[g]

            for t in range(n_tiles + 1):
                h1s = None
                if t < n_tiles:
                    g = t // OG
                    if t % OG == 0:
                        # output buffer for this group of row-tiles
                        # (rows: pl0, pl1, baseline, action — at partition 0)
                        ob = opool.tile([1, 4, OG * NT], F32, tag="ob")
                        et8 = opool.tile([1, OG * NT], F32, tag="eps8")
                        nc.sync.dma_start(et8[:],
                                          eps_d[:, ts(g, OG * NT)])
                        obs[g] = (ob, et8)
                    xt_t = xpool.tile([128, 4, NT], BF, tag="xt")
                    nc.sync.dma_start(xt_t[:], xt_d[:, :, ts(t, NT)])

                    # fc1: h1T chunks of 100
                    h1s = []
                    for m, (m0, mw) in enumerate(M1):
                        ps = ppool1.tile([mw, NT], F32, tag="ps1")
                        for k in range(4):
                            nc.tensor.matmul(
                                ps[:],
                                w1_sb[:, k, ds(m0, mw)],
                                xt_t[:, k, :],
                                start=(k == 0),
                                stop=(k == 3),
                            )
                        hs = h1pool.tile([mw, NT], BF, tag=f"h1_{m}",
                                         name=f"h1_{m}")
                        # relu(psum + b1) on DVE: (in + bias) max 0
                        nc.vector.tensor_scalar(
                            hs[:], ps[:], b1_sb[0:mw, m, :], 0.0,
                            ALU.add, ALU.max
                        )
                        h1s.append(hs)

                if pending is not None:
                    emit_head_epilogue(*pending)
                    pending = None

                if t < n_tiles:
                    # fc2: h2T chunks {128, 128, 44+zeros+[cr;la]}; the m=2
                    # chunk goes first so its multi-engine assembly (relu +
                    # rwla DMA + clip + memsets) finishes before the head
                    # matmuls consume it
                    cores = [None, None, None]
                    for m in (2, 0, 1):
                        m0, mw = M2[m]
                        ps2 = ppool2.tile([mw, NT], F32, tag="ps2")
                        for k in range(4):
                            nc.tensor.matmul(
                                ps2[:],
                                w2_sb[0:M1[k][1], k, ds(m0, mw)],
                                h1s[k][:],
                                start=(k == 0),
                                stop=(k == 3),
                            )
                        if m < 2:
                            cm = cpool.tile([128, NT], BF, tag="c")
                            nc.scalar.activation(cm[:], ps2[:], AF.Relu,
                                                 bias=b2_sb[0:mw, m, :])
                        else:
                            cm = cpool.tile([98, NT], BF, tag="c2")
                            nc.gpsimd.memset(cm[32:64, :], 0.0)
                            nc.gpsimd.memset(cm[64:96, :], 0.0)
                            nc.scalar.activation(cm[0:mw, :], ps2[:], AF.Relu,
                                                 bias=b2_sb[0:mw, m, :])
                            nc.sync.dma_start(cm[96:98, :],
                                              rwla_d[:, ts(t, NT)])
                            nc.vector.tensor_scalar(
                                cm[96:97, :], cm[96:97, :], -1.0, 1.0,
                                ALU.max, ALU.min)
                        cores[m] = cm
                    pending = (cores, t)

    nc.compile()
    return nc


def host_prep(frame, reward, last_action, eps, W1, b1, W2, b2, Wp, bp, Wb, bb,
              rows=R, n_cores=N_CORES):
    """Shard + lay out inputs for the device program. Returns in_maps."""
    frame = np.asarray(frame, np.float32).reshape(TB, OBS)
    reward = np.asarray(reward, np.float32).reshape(TB)
    la = np.asarray(last_action).reshape(TB).astype(BF16)
    eps = np.asarray(eps, np.float32).reshape(TB)

    W1 = np.asarray(W1, np.float32)
    W2 = np.asarray(W2, np.float32)
    b1 = np.asarray(b1, np.float32)
    b2 = np.asarray(b2, np.float32)
    Wp = np.asarray(Wp, np.float32)
    bp = np.asarray(bp, np.float32)
    Wb = np.asarray(Wb, np.float32)
    bb = np.asarray(bb, np.float32)

    w1_h = np.ascontiguousarray(
        W1.T.reshape(4, 128, 400).transpose(1, 0, 2)).astype(BF16)
    w2_h = np.ascontiguousarray(
        W2.T.reshape(4, 100, 300).transpose(1, 0, 2)).astype(BF16)
    # head weights: columns 0/32/64 of a zero-padded 65-wide matrix hold
    # (mu, sigma, baseline); contraction rows follow the fc2 chunking
    # {128, 128, 44} with rows 44..95 zero and [cr; la] weights at 96/97
    Wh65 = np.zeros((302, 65), np.float32)
    Wh65[:, 0] = Wp[0]
    Wh65[:, 32] = Wp[1]
    Wh65[:, 64] = Wb[0]
    wh_h = np.zeros((128, 3, 65), np.float32)
    wh_h[:, 0, :] = Wh65[0:128]
    wh_h[:, 1, :] = Wh65[128:256]
    wh_h[0:44, 2, :] = Wh65[256:300]
    wh_h[96:98, 2, :] = Wh65[300:302]
    wh_h = wh_h.astype(BF16)
    b1_h = np.ascontiguousarray(b1.reshape(4, 100).T)
    b2_h = np.zeros((128, 3), np.float32)
    b2_h[0:128, 0] = b2[0:128]
    b2_h[0:128, 1] = b2[128:256]
    b2_h[0:44, 2] = b2[256:300]
    bh_h = np.zeros((65, 1), np.float32)
    bh_h[0, 0] = bp[0]
    bh_h[32, 0] = bp[1]
    bh_h[64, 0] = bb[0]

    in_maps = []
    for c in range(n_cores):
        sl = slice(c * rows, (c + 1) * rows)
        xt = np.ascontiguousarray(
            frame[sl].T.reshape(4, 128, rows).transpose(1, 0, 2)).astype(BF16)
        rwla = np.stack([reward[sl].astype(BF16), la[sl]], axis=0)
        in_maps.append({
            "xt": xt,
            "rwla": rwla,
            "eps": eps[sl].reshape(1, rows),
            "w1": w1_h, "w2": w2_h, "wh": wh_h,
            "b1": b1_h, "b2": b2_h, "bh": bh_h,
        })
    return in_maps


def assemble_out(per_core_outs):
    """[4, R] per core (rows: pl0, pl1, baseline, action) -> [T, B, 4]."""
    outs = []
    for o in per_core_outs:
        outs.append(np.asarray(o).T.reshape(-1, B, 4))
    return np.ascontiguousarray(
        np.concatenate(outs, axis=0).astype(np.float32))


_NC_CACHE = {}


def kernel(**inputs) -> np.ndarray:
    in_maps = host_prep(**inputs)
    if R not in _NC_CACHE:
        _NC_CACHE[R] = build_bass(R)
    nc = _NC_CACHE[R]
    res = run_bass_kernel_spmd(nc, in_maps, core_ids=list(range(N_CORES)))
    return assemble_out([res.results[c]["out"] for c in range(N_CORES)])



# revision 3
# speedup vs baseline: 1.1446x; 1.1446x over previous
"""Trainium2 Bass kernel for nn_AutoPruneNet — fp8 DoubleRow version.

Math (per row r of TB = T*B rows):
    h1 = relu(x @ W1.T + b1)            x: [512], h1: [400]
    h2 = relu(h1 @ W2.T + b2)           h2: [300]
    core = [h2, clip(reward,-1,1), last_action]   [302]
    pl = sigmoid(core @ Wp.T + bp)      [2]  (mu, sigma)
    baseline = core @ Wb.T + bb         [1]
    action = pl0 + pl1 * eps
    out[r] = [pl0, pl1, baseline, action]

Distribution: pure data parallel, TB rows split contiguously across 8 cores
(16384 rows each); weights replicated.

Design vs the bf16 baseline (249.6us): the kernel is tensor-engine bound, so
all three matmul layers run in fp8e4m3 with DoubleRow perf mode (2 contraction
chunks per stream):
  - fc1: 4 m-chunks x 2 DR streams (contraction 512 = 2x(128+128))
  - fc2: 3 m-chunks x 2 DR streams (contraction 400 = 2x(100+100))
  - heads: 1 DR stream (h2[0:256]) + row-packed Ki=44 (h2[256:300]) and Ki=3
    ([cr, la, 1]) streams at disjoint row groups (concurrent on the PE).
    The constant-1 row folds all three head biases into the matmul, so the
    baseline head output needs no engine op at all (DMA'd straight from PSUM).
Head outputs sit at stationary cols 0 (mu), 1 (sigma), 64 (baseline) so one
sigmoid ACT op covers mu+sigma. Epilogues alternate ACT/DVE per chunk to stay
off the critical path; action = pl0 + pl1*eps is batched per 4-tile group.
fp8 weight quantization error analysis gives ~1e-2 max rel err vs the 2e-2
gate (weights/e4m3 subnormals contribute harmlessly small absolute error).
"""
import sys
import types

import numpy as np
import ml_dtypes

import concourse.bacc as bacc
import concourse.bass as bass
import concourse.mybir as mybir
import concourse.tile as tile
from concourse.bass import ds, ts
from concourse.bass_utils import run_bass_kernel_spmd


def _install_ntff_hook_shim():
    """Provide the optional antenv.axon_hooks module if the image lacks it,
    so a BASS_TRACE env var in the caller can't crash run_bass_kernel_spmd.
    Registers the real NTFF profile hook when the axon .so supports it."""
    try:
        import antenv.axon_hooks  # noqa: F401
        return
    except Exception:
        pass
    try:
        import antenv
    except Exception:
        return
    mod = types.ModuleType("antenv.axon_hooks")
    state = {"hook": None}
    mod.set_axon_ntff_profile_hook = lambda h: state.__setitem__("hook", h)
    mod.get_axon_ntff_profile_hook = lambda: state["hook"]
    sys.modules["antenv.axon_hooks"] = mod
    antenv.axon_hooks = mod
    try:
        from trn_agent_boot.trn_boot import _ntff_profile_via_ctypes
        mod.set_axon_ntff_profile_hook(
            _ntff_profile_via_ctypes('/opt/axon/libaxon_pjrt.so'))
    except Exception:
        pass


_install_ntff_hook_shim()

FP8 = ml_dtypes.float8_e4m3   # IEEE-style e4m3: max 240, infinities — TRN FP8_EXP4

N_CORES = 8
T, B, OBS = 64, 2048, 512
H1, H2 = 400, 300
TB = T * B
R = TB // N_CORES       # rows per core
NT = 512                # rows per row-tile (matmul moving dim)
OG = 4                  # row-tiles per output-DMA group

F32 = mybir.dt.float32
F8 = mybir.dt.float8e4
AF = mybir.ActivationFunctionType
ALU = mybir.AluOpType
DR = mybir.MatmulPerfMode.DoubleRow

# fc2 output (h2) chunks: {128, 128, 44}
M2 = [(0, 128), (128, 128), (256, 44)]


def build_bass(rows: int):
    """Build the per-core Bass program for `rows` rows (rows % (NT*OG) == 0)."""
    assert rows % (NT * OG) == 0
    n_tiles = rows // NT

    nc = bacc.Bacc("TRN2", target_bir_lowering=False, debug=False)

    # x pre-tiled by row-tile: [128, n_tiles, 4, NT] so each tile DMA is
    # contiguous 2KB per partition
    xt_d = nc.dram_tensor("xt", [128, n_tiles, 4, NT], F8, kind="ExternalInput")
    # rows: clip(reward), last_action, ones (bias row for the head matmul)
    rwla_d = nc.dram_tensor("rwla", [3, rows], F8, kind="ExternalInput")
    eps_d = nc.dram_tensor("eps", [1, rows], F32, kind="ExternalInput")
    w1_d = nc.dram_tensor("w1", [128, 4, 400], F8, kind="ExternalInput")
    w2_d = nc.dram_tensor("w2", [100, 4, 304], F8, kind="ExternalInput")
    whp_d = nc.dram_tensor("whp", [128, 2, 80], F8, kind="ExternalInput")
    whc_d = nc.dram_tensor("whc", [67, 80], F8, kind="ExternalInput")
    b1_d = nc.dram_tensor("b1", [100, 4], F32, kind="ExternalInput")
    b2_d = nc.dram_tensor("b2", [128, 3], F32, kind="ExternalInput")
    out_d = nc.dram_tensor("out", [4, rows], F32, kind="ExternalOutput")

    with tile.TileContext(nc) as tc:
        with (
            tc.tile_pool(name="w", bufs=1) as wpool,
            tc.tile_pool(name="x", bufs=3) as xpool,
            tc.tile_pool(name="h1", bufs=3) as h1pool,
            tc.tile_pool(name="core", bufs=3) as cpool,
            tc.tile_pool(name="g", bufs=2) as gpool,
            tc.tile_pool(name="ps1", bufs=4, space="PSUM") as ppool1,
            tc.tile_pool(name="ps2", bufs=2, space="PSUM") as ppool2,
            tc.tile_pool(name="ps3", bufs=2, space="PSUM") as ppool3,
        ):
            w1_sb = wpool.tile([128, 4, 400], F8, tag="w1")
            nc.scalar.dma_start(w1_sb[:], w1_d[:])
            w2_sb = wpool.tile([100, 4, 304], F8, tag="w2")
            nc.scalar.dma_start(w2_sb[:], w2_d[:])
            whp_sb = wpool.tile([128, 2, 80], F8, tag="whp")
            nc.scalar.dma_start(whp_sb[:], whp_d[:])
            whc_sb = wpool.tile([67, 80], F8, tag="whc")
            nc.scalar.dma_start(whc_sb[:], whc_d[:])
            b1_sb = wpool.tile([100, 4, 1], F32, tag="b1")
            nc.scalar.dma_start(b1_sb[:], b1_d[:])
            b2_sb = wpool.tile([128, 3, 1], F32, tag="b2")
            nc.scalar.dma_start(b2_sb[:], b2_d[:])

            # group -> (st [2, OG*NT] (pl0/pl1), blt, se, actt, epst)
            groups = {}
            pending = None  # (cp, c2, t) awaiting head matmuls + epilogue

            def emit_heads(cp, c2, t):
                g, ti = divmod(t, OG)
                st, blt, se, actt, epst = groups[g]
                # psum: row 0 = mu_pre+bp0, 32 = sigma_pre+bp1, 64 = baseline
                # (engine AP starts must be 0/32/64/96; one sigmoid op over
                # partitions 0..32 covers mu+sigma — junk lanes are free)
                psh = ppool3.tile([65, NT], F32, tag="ps3")
                nc.tensor.matmul(psh[:], whp_sb[:, :, 0:65], cp[:, :, :],
                                 start=True, stop=False, perf_mode=DR)
                nc.tensor.matmul(psh[:], whc_sb[0:44, 0:65], c2[0:44, :],
                                 start=False, stop=False)
                nc.tensor.matmul(psh[:], whc_sb[64:67, 0:65], c2[64:67, :],
                                 start=False, stop=True)
                sl = ds(ti * NT, NT)
                nc.scalar.activation(st[:, sl], psh[0:33, :], AF.Sigmoid)
                # baseline: bias came in through the matmul, just move to SBUF
                nc.scalar.copy(blt[:, sl], psh[64:65, :])
                if ti == OG - 1:
                    gsl = ts(g, OG * NT)
                    # DVE tensor_tensor needs equal start partitions: move
                    # pl1 (partition 32) down to partition 0 via SBUF DMA
                    st1 = gpool.tile([1, OG * NT], F32, tag="st1")
                    nc.sync.dma_start(st1[:], st[32:33, :])
                    nc.vector.tensor_mul(se[:], st1[:], epst[:])
                    nc.vector.tensor_add(actt[:], st[0:1, :], se[:])
                    nc.sync.dma_start(out_d[0:1, gsl], st[0:1, :])
                    nc.sync.dma_start(out_d[1:2, gsl], st1[:])
                    nc.sync.dma_start(out_d[2:3, gsl], blt[:])
                    nc.sync.dma_start(out_d[3:4, gsl], actt[:])
                    del groups[g]

            for t in range(n_tiles + 1):
                h1_t = cp_t = c2_t = None
                if t < n_tiles:
                    g = t // OG
                    if t % OG == 0:
                        st = gpool.tile([33, OG * NT], F32, tag="st")
                        blt = gpool.tile([1, OG * NT], F32, tag="blt")
                        se = gpool.tile([1, OG * NT], F32, tag="se")
                        actt = gpool.tile([1, OG * NT], F32, tag="actt")
                        epst = gpool.tile([1, OG * NT], F32, tag="epst")
                        nc.sync.dma_start(epst[:], eps_d[:, ts(g, OG * NT)])
                        groups[g] = (st, blt, se, actt, epst)
                    xt_t = xpool.tile([128, 4, NT], F8, tag="xt")
                    nc.sync.dma_start(xt_t[:], xt_d[:, t, :, :])
                    # core chunk 2: relu rows 0..43 (ACT), [cr, la, 1] rows
                    # 64..66 (DMA); rows 44..63 never touched
                    c2_t = cpool.tile([67, NT], F8, tag="c2")
                    nc.sync.dma_start(c2_t[64:67, :], rwla_d[:, ts(t, NT)])

                    # fc1: 4 m-chunks of 100, each 2 DoubleRow streams
                    h1_t = h1pool.tile([100, 4, NT], F8, tag="h1")
                    for m in range(4):
                        ps = ppool1.tile([100, NT], F32, tag="ps1")
                        for p in range(2):
                            nc.tensor.matmul(
                                ps[:],
                                w1_sb[:, 2 * p:2 * p + 2, ds(100 * m, 100)],
                                xt_t[:, 2 * p:2 * p + 2, :],
                                start=(p == 0), stop=(p == 1), perf_mode=DR,
                            )
                        # relu(psum + b1) -> fp8; alternate engines so each
                        # chunk's epilogue hides under later matmul streams
                        if m % 2 == 0:
                            nc.scalar.activation(h1_t[:, m, :], ps[:], AF.Relu,
                                                 bias=b1_sb[:, m, :])
                        else:
                            nc.vector.tensor_scalar(
                                h1_t[:, m, :], ps[:], b1_sb[:, m, :], 0.0,
                                ALU.add, ALU.max)

                if pending is not None:
                    emit_heads(*pending)
                    pending = None

                if t < n_tiles:
                    # fc2: m-chunks {128, 128, 44}, each 2 DoubleRow streams
                    cp_t = cpool.tile([128, 2, NT], F8, tag="cp")
                    for m in range(3):
                        m0, mw = M2[m]
                        ps2 = ppool2.tile([mw, NT], F32, tag="ps2")
                        for p in range(2):
                            nc.tensor.matmul(
                                ps2[:],
                                w2_sb[:, 2 * p:2 * p + 2, ds(m0, mw)],
                                h1_t[:, 2 * p:2 * p + 2, :],
                                start=(p == 0), stop=(p == 1), perf_mode=DR,
                            )
                        if m < 2:
                            nc.vector.tensor_scalar(
                                cp_t[:, m, :], ps2[:], b2_sb[0:mw, m, :], 0.0,
                                ALU.add, ALU.max)
                        else:
                            nc.scalar.activation(c2_t[0:44, :], ps2[:], AF.Relu,
                                                 bias=b2_sb[0:44, 2, :])
                    pending = (cp_t, c2_t, t)

    nc.compile()
    return nc


def host_prep(frame, reward, last_action, eps, W1, b1, W2, b2, Wp, bp, Wb, bb,
              rows=R, n_cores=N_CORES):
    """Shard + lay out inputs for the device program. Returns in_maps."""
    n_tiles = rows // NT
    frame = np.asarray(frame, np.float32).reshape(TB, OBS)
    cr = np.clip(np.asarray(reward, np.float32).reshape(TB), -1.0, 1.0)
    la = np.asarray(last_action).reshape(TB).astype(np.float32)
    eps = np.asarray(eps, np.float32).reshape(TB)

    W1 = np.asarray(W1, np.float32)
    W2 = np.asarray(W2, np.float32)
    b1 = np.asarray(b1, np.float32)
    b2 = np.asarray(b2, np.float32)
    Wp = np.asarray(Wp, np.float32)
    bp = np.asarray(bp, np.float32)
    Wb = np.asarray(Wb, np.float32)
    bb = np.asarray(bb, np.float32)

    w1_h = np.ascontiguousarray(
        W1.T.reshape(4, 128, 400).transpose(1, 0, 2)).astype(FP8)
    w2_h = np.zeros((100, 4, 304), np.float32)
    w2_h[:, :, 0:300] = W2.T.reshape(4, 100, 300).transpose(1, 0, 2)
    w2_h = w2_h.astype(FP8)
    # head weights: stationary cols 0 = mu, 32 = sigma, 64 = baseline
    whp_h = np.zeros((128, 2, 80), np.float32)
    whc_h = np.zeros((67, 80), np.float32)
    for col, w_row, b_val in ((0, Wp[0], bp[0]), (32, Wp[1], bp[1]),
                              (64, Wb[0], bb[0])):
        whp_h[:, 0, col] = w_row[0:128]
        whp_h[:, 1, col] = w_row[128:256]
        whc_h[0:44, col] = w_row[256:300]
        whc_h[64, col] = w_row[300]      # cr weight
        whc_h[65, col] = w_row[301]      # la weight
        whc_h[66, col] = b_val           # bias via the constant-1 row
    whp_h = whp_h.astype(FP8)
    whc_h = whc_h.astype(FP8)
    b1_h = np.ascontiguousarray(b1.reshape(4, 100).T)
    b2_h = np.zeros((128, 3), np.float32)
    b2_h[0:128, 0] = b2[0:128]
    b2_h[0:128, 1] = b2[128:256]
    b2_h[0:44, 2] = b2[256:300]

    in_maps = []
    for c in range(n_cores):
        sl = slice(c * rows, (c + 1) * rows)
        xt = np.ascontiguousarray(
            frame[sl].reshape(n_tiles, NT, 4, 128).transpose(3, 0, 2, 1)
        ).astype(FP8)
        rwla = np.stack([cr[sl], la[sl], np.ones(rows, np.float32)],
                        axis=0).astype(FP8)
        in_maps.append({
            "xt": xt,
            "rwla": rwla,
            "eps": eps[sl].reshape(1, rows),
            "w1": w1_h, "w2": w2_h, "whp": whp_h, "whc": whc_h,
            "b1": b1_h, "b2": b2_h,
        })
    return in_maps


def assemble_out(per_core_outs):
    """[4, R] per core (rows: pl0, pl1, baseline, action) -> [T, B, 4]."""
    outs = []
    for o in per_core_outs:
        outs.append(np.asarray(o).T.reshape(-1, B, 4))
    return np.ascontiguousarray(
        np.concatenate(outs, axis=0).astype(np.float32))


_NC_CACHE = {}


def kernel(**inputs) -> np.ndarray:
    in_maps = host_prep(**inputs)
    if R not in _NC_CACHE:
        _NC_CACHE[R] = build_bass(R)
    nc = _NC_CACHE[R]
    res = run_bass_kernel_spmd(nc, in_maps, core_ids=list(range(N_CORES)))
    return assemble_out([res.results[c]["out"] for c in range(N_CORES)])


# revision 4
# speedup vs baseline: 1.3760x; 1.2021x over previous
"""Trainium2 Bass kernel for nn_AutoPruneNet — fp8 DoubleRow version.

Math (per row r of TB = T*B rows):
    h1 = relu(x @ W1.T + b1)            x: [512], h1: [400]
    h2 = relu(h1 @ W2.T + b2)           h2: [300]
    core = [h2, clip(reward,-1,1), last_action]   [302]
    pl = sigmoid(core @ Wp.T + bp)      [2]  (mu, sigma)
    baseline = core @ Wb.T + bb         [1]
    action = pl0 + pl1 * eps
    out[r] = [pl0, pl1, baseline, action]

Distribution: pure data parallel, TB rows split contiguously across 8 cores
(16384 rows each); weights replicated.

Design vs the bf16 baseline (249.6us): the kernel is tensor-engine bound, so
all three matmul layers run in fp8e4m3 with DoubleRow perf mode (2 contraction
chunks per stream):
  - fc1: 4 m-chunks x 2 DR streams (contraction 512 = 2x(128+128))
  - fc2: 3 m-chunks x 2 DR streams (contraction 400 = 2x(100+100))
  - heads: 1 DR stream (h2[0:256]) + row-packed Ki=44 (h2[256:300]) and Ki=3
    ([cr, la, 1]) streams at disjoint row groups (concurrent on the PE).
    The constant-1 row folds all three head biases into the matmul, so the
    baseline head output needs no engine op at all (DMA'd straight from PSUM).
Head outputs sit at stationary cols 0 (mu), 1 (sigma), 64 (baseline) so one
sigmoid ACT op covers mu+sigma. Epilogues alternate ACT/DVE per chunk to stay
off the critical path; action = pl0 + pl1*eps is batched per 4-tile group.
fp8 weight quantization error analysis gives ~1e-2 max rel err vs the 2e-2
gate (weights/e4m3 subnormals contribute harmlessly small absolute error).
"""
import sys
import types

import numpy as np
import ml_dtypes

import concourse.bacc as bacc
import concourse.bass as bass
import concourse.mybir as mybir
import concourse.tile as tile
from concourse.bass import ds, ts
from concourse.bass_utils import run_bass_kernel_spmd


def _install_ntff_hook_shim():
    """Provide the optional antenv.axon_hooks module if the image lacks it,
    so a BASS_TRACE env var in the caller can't crash run_bass_kernel_spmd.
    Registers the real NTFF profile hook when the axon .so supports it."""
    try:
        import antenv.axon_hooks  # noqa: F401
        return
    except Exception:
        pass
    try:
        import antenv
    except Exception:
        return
    mod = types.ModuleType("antenv.axon_hooks")
    state = {"hook": None}
    mod.set_axon_ntff_profile_hook = lambda h: state.__setitem__("hook", h)
    mod.get_axon_ntff_profile_hook = lambda: state["hook"]
    sys.modules["antenv.axon_hooks"] = mod
    antenv.axon_hooks = mod
    try:
        from trn_agent_boot.trn_boot import _ntff_profile_via_ctypes
        mod.set_axon_ntff_profile_hook(
            _ntff_profile_via_ctypes('/opt/axon/libaxon_pjrt.so'))
    except Exception:
        pass


_install_ntff_hook_shim()

FP8 = ml_dtypes.float8_e4m3   # IEEE-style e4m3: max 240, infinities — TRN FP8_EXP4

N_CORES = 8
T, B, OBS = 64, 2048, 512
H1, H2 = 400, 300
TB = T * B
R = TB // N_CORES       # rows per core
NT = 512                # rows per row-tile (matmul moving dim)
OG = 4                  # row-tiles per output-DMA group

F32 = mybir.dt.float32
F8 = mybir.dt.float8e4
AF = mybir.ActivationFunctionType
ALU = mybir.AluOpType
DR = mybir.MatmulPerfMode.DoubleRow

# fc2 output (h2) chunks: {128, 128, 44}
M2 = [(0, 128), (128, 128), (256, 44)]


def build_bass(rows: int):
    """Build the per-core Bass program for `rows` rows (rows % (NT*OG) == 0)."""
    assert rows % (NT * OG) == 0
    n_tiles = rows // NT

    nc = bacc.Bacc("TRN2", target_bir_lowering=False, debug=False)

    # x pre-tiled by row-tile: [128, n_tiles, 4, NT] so each tile DMA is
    # contiguous 2KB per partition
    xt_d = nc.dram_tensor("xt", [128, n_tiles, 4, NT], F8, kind="ExternalInput")
    # rows: clip(reward), last_action, ones (bias row for the head matmul)
    rwla_d = nc.dram_tensor("rwla", [3, rows], F8, kind="ExternalInput")
    eps_d = nc.dram_tensor("eps", [1, rows], F32, kind="ExternalInput")
    w1_d = nc.dram_tensor("w1", [128, 4, 400], F8, kind="ExternalInput")
    w2_d = nc.dram_tensor("w2", [100, 4, 304], F8, kind="ExternalInput")
    whp_d = nc.dram_tensor("whp", [128, 2, 80], F8, kind="ExternalInput")
    whc_d = nc.dram_tensor("whc", [67, 80], F8, kind="ExternalInput")
    b1_d = nc.dram_tensor("b1", [100, 4], F32, kind="ExternalInput")
    b2_d = nc.dram_tensor("b2", [128, 3], F32, kind="ExternalInput")
    out_d = nc.dram_tensor("out", [4, rows], F32, kind="ExternalOutput")

    with tile.TileContext(nc) as tc:
        with (
            tc.tile_pool(name="w", bufs=1) as wpool,
            tc.tile_pool(name="x", bufs=4) as xpool,
            tc.tile_pool(name="h1", bufs=3) as h1pool,
            tc.tile_pool(name="core", bufs=3) as cpool,
            tc.tile_pool(name="g", bufs=2) as gpool,
            tc.tile_pool(name="ps1", bufs=4, space="PSUM") as ppool1,
            tc.tile_pool(name="ps2", bufs=2, space="PSUM") as ppool2,
            tc.tile_pool(name="ps3", bufs=2, space="PSUM") as ppool3,
        ):
            w1_sb = wpool.tile([128, 4, 400], F8, tag="w1")
            nc.scalar.dma_start(w1_sb[:], w1_d[:])
            w2_sb = wpool.tile([100, 4, 304], F8, tag="w2")
            nc.scalar.dma_start(w2_sb[:], w2_d[:])
            whp_sb = wpool.tile([128, 2, 80], F8, tag="whp")
            nc.scalar.dma_start(whp_sb[:], whp_d[:])
            whc_sb = wpool.tile([67, 80], F8, tag="whc")
            nc.scalar.dma_start(whc_sb[:], whc_d[:])
            b1_sb = wpool.tile([100, 4, 1], F32, tag="b1")
            nc.scalar.dma_start(b1_sb[:], b1_d[:])
            b2_sb = wpool.tile([128, 3, 1], F32, tag="b2")
            nc.scalar.dma_start(b2_sb[:], b2_d[:])

            # group -> (st [2, OG*NT] (pl0/pl1), blt, se, actt, epst)
            groups = {}
            pending = None  # (cp, c2, t) awaiting head matmuls + epilogue

            def emit_heads(cp, c2, t):
                g, ti = divmod(t, OG)
                st, blt, se, actt, epst = groups[g]
                # psum: row 0 = mu_pre+bp0, 32 = sigma_pre+bp1, 64 = baseline
                # (engine AP starts must be 0/32/64/96; one sigmoid op over
                # partitions 0..32 covers mu+sigma — junk lanes are free)
                psh = ppool3.tile([65, NT], F32, tag="ps3")
                nc.tensor.matmul(psh[:], whp_sb[:, :, 0:65], cp[:, :, :],
                                 start=True, stop=False, perf_mode=DR)
                nc.tensor.matmul(psh[:], whc_sb[0:44, 0:65], c2[0:44, :],
                                 start=False, stop=False)
                nc.tensor.matmul(psh[:], whc_sb[64:67, 0:65], c2[64:67, :],
                                 start=False, stop=True)
                sl = ds(ti * NT, NT)
                nc.scalar.activation(st[:, sl], psh[0:33, :], AF.Sigmoid)
                # baseline: bias came in through the matmul, just move to SBUF
                nc.scalar.copy(blt[:, sl], psh[64:65, :])
                if ti == OG - 1:
                    # The whole group tail lives on the (otherwise idle)
                    # GPSIMD engine + its DMA queue: output DMAs wait on the
                    # se/act chain, and on the Sync queue they'd head-of-line
                    # block the next tiles' input DMAs (measured 10-25us).
                    gsl = ts(g, OG * NT)
                    # tensor_tensor needs equal start partitions: move pl1
                    # (partition 32) down to partition 0 via SBUF DMA
                    st1 = gpool.tile([1, OG * NT], F32, tag="st1")
                    nc.gpsimd.dma_start(st1[:], st[32:33, :])
                    nc.gpsimd.tensor_mul(se[:], st1[:], epst[:])
                    nc.gpsimd.tensor_add(actt[:], st[0:1, :], se[:])
                    nc.gpsimd.dma_start(out_d[0:1, gsl], st[0:1, :])
                    nc.gpsimd.dma_start(out_d[1:2, gsl], st1[:])
                    nc.gpsimd.dma_start(out_d[2:3, gsl], blt[:])
                    nc.gpsimd.dma_start(out_d[3:4, gsl], actt[:])
                    del groups[g]

            for t in range(n_tiles + 1):
                h1_t = cp_t = c2_t = None
                if t < n_tiles:
                    g = t // OG
                    if t % OG == 0:
                        st = gpool.tile([33, OG * NT], F32, tag="st")
                        blt = gpool.tile([1, OG * NT], F32, tag="blt")
                        se = gpool.tile([1, OG * NT], F32, tag="se")
                        actt = gpool.tile([1, OG * NT], F32, tag="actt")
                        epst = gpool.tile([1, OG * NT], F32, tag="epst")
                        nc.sync.dma_start(epst[:], eps_d[:, ts(g, OG * NT)])
                        groups[g] = (st, blt, se, actt, epst)
                    xt_t = xpool.tile([128, 4, NT], F8, tag="xt")
                    nc.sync.dma_start(xt_t[:], xt_d[:, t, :, :])
                    # core chunk 2: relu rows 0..43 (ACT), [cr, la, 1] rows
                    # 64..66 (DMA); rows 44..63 never touched
                    c2_t = cpool.tile([67, NT], F8, tag="c2")
                    nc.sync.dma_start(c2_t[64:67, :], rwla_d[:, ts(t, NT)])

                    # fc1: 4 m-chunks of 100, each 2 DoubleRow streams
                    h1_t = h1pool.tile([100, 4, NT], F8, tag="h1")
                    for m in range(4):
                        ps = ppool1.tile([100, NT], F32, tag="ps1")
                        for p in range(2):
                            nc.tensor.matmul(
                                ps[:],
                                w1_sb[:, 2 * p:2 * p + 2, ds(100 * m, 100)],
                                xt_t[:, 2 * p:2 * p + 2, :],
                                start=(p == 0), stop=(p == 1), perf_mode=DR,
                            )
                        # relu(psum + b1) -> fp8; alternate engines so each
                        # chunk's epilogue hides under later matmul streams
                        if m % 2 == 0:
                            nc.scalar.activation(h1_t[:, m, :], ps[:], AF.Relu,
                                                 bias=b1_sb[:, m, :])
                        else:
                            nc.vector.tensor_scalar(
                                h1_t[:, m, :], ps[:], b1_sb[:, m, :], 0.0,
                                ALU.add, ALU.max)

                if pending is not None:
                    emit_heads(*pending)
                    pending = None

                if t < n_tiles:
                    # fc2: m-chunks {128, 128, 44}, each 2 DoubleRow streams
                    cp_t = cpool.tile([128, 2, NT], F8, tag="cp")
                    for m in range(3):
                        m0, mw = M2[m]
                        ps2 = ppool2.tile([mw, NT], F32, tag="ps2")
                        for p in range(2):
                            nc.tensor.matmul(
                                ps2[:],
                                w2_sb[:, 2 * p:2 * p + 2, ds(m0, mw)],
                                h1_t[:, 2 * p:2 * p + 2, :],
                                start=(p == 0), stop=(p == 1), perf_mode=DR,
                            )
                        if m < 2:
                            nc.vector.tensor_scalar(
                                cp_t[:, m, :], ps2[:], b2_sb[0:mw, m, :], 0.0,
                                ALU.add, ALU.max)
                        else:
                            nc.scalar.activation(c2_t[0:44, :], ps2[:], AF.Relu,
                                                 bias=b2_sb[0:44, 2, :])
                    pending = (cp_t, c2_t, t)

    nc.compile()
    return nc


def host_prep(frame, reward, last_action, eps, W1, b1, W2, b2, Wp, bp, Wb, bb,
              rows=R, n_cores=N_CORES):
    """Shard + lay out inputs for the device program. Returns in_maps."""
    n_tiles = rows // NT
    frame = np.asarray(frame, np.float32).reshape(TB, OBS)
    cr = np.clip(np.asarray(reward, np.float32).reshape(TB), -1.0, 1.0)
    la = np.asarray(last_action).reshape(TB).astype(np.float32)
    eps = np.asarray(eps, np.float32).reshape(TB)

    W1 = np.asarray(W1, np.float32)
    W2 = np.asarray(W2, np.float32)
    b1 = np.asarray(b1, np.float32)
    b2 = np.asarray(b2, np.float32)
    Wp = np.asarray(Wp, np.float32)
    bp = np.asarray(bp, np.float32)
    Wb = np.asarray(Wb, np.float32)
    bb = np.asarray(bb, np.float32)

    w1_h = np.ascontiguousarray(
        W1.T.reshape(4, 128, 400).transpose(1, 0, 2)).astype(FP8)
    w2_h = np.zeros((100, 4, 304), np.float32)
    w2_h[:, :, 0:300] = W2.T.reshape(4, 100, 300).transpose(1, 0, 2)
    w2_h = w2_h.astype(FP8)
    # head weights: stationary cols 0 = mu, 32 = sigma, 64 = baseline
    whp_h = np.zeros((128, 2, 80), np.float32)
    whc_h = np.zeros((67, 80), np.float32)
    for col, w_row, b_val in ((0, Wp[0], bp[0]), (32, Wp[1], bp[1]),
                              (64, Wb[0], bb[0])):
        whp_h[:, 0, col] = w_row[0:128]
        whp_h[:, 1, col] = w_row[128:256]
        whc_h[0:44, col] = w_row[256:300]
        whc_h[64, col] = w_row[300]      # cr weight
        whc_h[65, col] = w_row[301]      # la weight
        whc_h[66, col] = b_val           # bias via the constant-1 row
    whp_h = whp_h.astype(FP8)
    whc_h = whc_h.astype(FP8)
    b1_h = np.ascontiguousarray(b1.reshape(4, 100).T)
    b2_h = np.zeros((128, 3), np.float32)
    b2_h[0:128, 0] = b2[0:128]
    b2_h[0:128, 1] = b2[128:256]
    b2_h[0:44, 2] = b2[256:300]

    in_maps = []
    for c in range(n_cores):
        sl = slice(c * rows, (c + 1) * rows)
        xt = np.ascontiguousarray(
            frame[sl].reshape(n_tiles, NT, 4, 128).transpose(3, 0, 2, 1)
        ).astype(FP8)
        rwla = np.stack([cr[sl], la[sl], np.ones(rows, np.float32)],
                        axis=0).astype(FP8)
        in_maps.append({
            "xt": xt,
            "rwla": rwla,
            "eps": eps[sl].reshape(1, rows),
            "w1": w1_h, "w2": w2_h, "whp": whp_h, "whc": whc_h,
            "b1": b1_h, "b2": b2_h,
        })
    return in_maps


def assemble_out(per_core_outs):
    """[4, R] per core (rows: pl0, pl1, baseline, action) -> [T, B, 4]."""
    outs = []
    for o in per_core_outs:
        outs.append(np.asarray(o).T.reshape(-1, B, 4))
    return np.ascontiguousarray(
        np.concatenate(outs, axis=0).astype(np.float32))


_NC_CACHE = {}


def kernel(**inputs) -> np.ndarray:
    in_maps = host_prep(**inputs)
    if R not in _NC_CACHE:
        _NC_CACHE[R] = build_bass(R)
    nc = _NC_CACHE[R]
    res = run_bass_kernel_spmd(nc, in_maps, core_ids=list(range(N_CORES)))
    return assemble_out([res.results[c]["out"] for c in range(N_CORES)])
